# revision 15
# baseline (speedup 1.0000x reference)
"""Trainium2 Bass kernel for nn_CoscamLoss (hard-example-scaled masked CE loss).

Math: loss = mean_i [ logsumexp_j(out_ij) - out_{i,t_i} ] where
  out_ij = 16 * x_ij,  x_ij = hard ? 1.012*inp + 0.012 : inp,
  hard   = pos_cam_mask AND (inp >= gt_i),  gt_i = inp[i, t_i],
  and the target column is restored to gt_i (minus margin 0.1).

Device kernel computes, per row, s_i = sum_j exp(16*u*q - (K+16)) with
  u = inp + 1,  q = 1 + 0.012*pos  (q in {1.0, 1.012}).
This equals the true term except for pos=1 entries with inp in [-1, gt):
those are ~exp(16*(gt - rowmax)) below the row max, i.e. numerically
irrelevant (same approximation class as max(e0, pos*e1)).

Encoding: ONE fp16 tensor is shipped per element: u = fp16(inp + 1) with
the mask packed into magnitude-mantissa bits 2-3 (pattern 0b1100 = pos,
0b0000 = not pos; values rounded to the nearest fp16 consistent with the
pattern). On device the multiplier q is reconstructed with a 4x-mode
tensor_scalar:  q = bitcast_fp16((u & 0x000C) | 0x3C00) in {1.0,
1.01171875} (0x000C as an fp16 mantissa increment is exactly the fp16
hard-scale delta). Then v = u*q (2x tensor_tensor) and exp+row-accum on
the scalar engine. One designated chunk instead computes its exp on the
vector engine via the Schraudolph bit trick (i32 = A*v + B, bitcast to
f32, max(.,0) + accumulate) to offload the ACT bottleneck; encode snaps
u in [1.66, 1.76] to 1.5 so no bitcast can land in the -NaN window.
First/last row-blocks use a chunk-size ladder so the scalar engine
lights up earlier and drains later chunks sooner. The target-column
term, the log, and the mean are corrected on the host (O(B) work,
replicating the device arithmetic exactly).
Sharding: data-parallel over batch, 512 rows per core.
"""

import numpy as np

B, C = 4096, 16384
N_CORES = 8
ROWS = B // N_CORES  # 512 rows per core
P = 128              # SBUF partitions
RB = ROWS // P       # 4 row-blocks per core
FD = 4096            # max free-dim chunk along C
K = 100.0            # fixed log-sum-exp offset
SCALE = 16.0
MARGIN = 0.1
BIAS = -(K + SCALE)  # -116: exp(16*u*q + BIAS) = exp(16*w - K)
Q_HARD = float(np.float16(1.012))  # 1.01171875 = 1 + 12 * 2^-10

# chunk-size ladder per row-block (sums to C=16384 each)
CHUNKS_EDGE_HEAD = [1024, 1024, 2048, 4096, 4096, 4096]
CHUNKS_MID = [4096, 4096, 4096, 4096]
CHUNKS_EDGE_TAIL = [4096, 4096, 4096, 2048, 1024, 1024]
RB_CHUNKS = [CHUNKS_EDGE_HEAD, CHUNKS_MID, CHUNKS_MID, CHUNKS_EDGE_TAIL]

# Schraudolph fast-exp: exp(16*v - 116) ~ bitcast_f32(int32(A*v + B)),
# clamped below at 0. Centered to halve the one-sided PWL error.
LOG2E = 1.4426950408889634
FE_A = 193635248.0
FE_B = -338868736.0
# (rb, ci) chunks whose exp+sum runs on the vector engine instead of ACT
FASTEXP_CHUNKS = {(2, 2)}
# (rb, ci) chunks whose q-decode runs on gpsimd (Pool) instead of DVE
# (empty: the TensorScalarPtr opcode is not legal on Pool on core v3)
POOL_DECODE_CHUNKS = set()

_CACHE = {}


def _build():
    import concourse.bass as bass
    import concourse.bacc as bacc
    import concourse.mybir as mybir
    import concourse.tile as tile

    rb_n = RB

    nc = bacc.Bacc(None, target_bir_lowering=False)
    x = nc.dram_tensor("x", [ROWS, C], mybir.dt.float16, kind="ExternalInput")
    out = nc.dram_tensor("out", [P, rb_n], mybir.dt.float32, kind="ExternalOutput")

    x_r = x.rearrange("(rb p) c -> rb p c", p=P)

    Alu = mybir.AluOpType
    Act = mybir.ActivationFunctionType

    with tile.TileContext(nc) as tc:
        with (
            tc.tile_pool(name="io", bufs=6) as io,
            tc.tile_pool(name="work", bufs=3) as work,
            tc.tile_pool(name="scr", bufs=1) as scr,
            tc.tile_pool(name="accp", bufs=2) as accp,
            tc.tile_pool(name="outp", bufs=1) as outp,
        ):
            stats = outp.tile([P, rb_n], mybir.dt.float32)
            bias_t = outp.tile([P, 1], mybir.dt.float32, tag="bias")
            nc.vector.memset(bias_t, BIAS)
            for rb in range(rb_n):
                sizes = RB_CHUNKS[rb]
                nchunk = len(sizes)
                parts = accp.tile([P, nchunk], mybir.dt.float32, tag=f"parts{nchunk}")
                off = 0
                for ci, sz in enumerate(sizes):
                    xt_f = io.tile([P, FD], mybir.dt.float16, tag="xt")
                    xt = xt_f[:, :sz]
                    nc.sync.dma_start(out=xt, in_=x_r[rb, :, off : off + sz])
                    qt_f = work.tile([P, FD], mybir.dt.uint16, tag="qt")
                    qt = qt_f[:, :sz]
                    dec_eng = nc.gpsimd if (rb, ci) in POOL_DECODE_CHUNKS else nc.vector
                    # q = (u & 0x000C) | 0x3C00 : fp16 {1.0, 1.01171875}
                    dec_eng.tensor_scalar(
                        out=qt, in0=xt.bitcast(mybir.dt.uint16),
                        scalar1=12, scalar2=15360,
                        op0=Alu.bitwise_and, op1=Alu.bitwise_or,
                    )
                    vt_f = work.tile([P, FD], mybir.dt.float16, tag="vt")
                    vt = vt_f[:, :sz]
                    nc.vector.tensor_tensor(
                        out=vt, in0=xt, in1=qt.bitcast(mybir.dt.float16),
                        op=Alu.mult,
                    )
                    if (rb, ci) in FASTEXP_CHUNKS:
                        i32t_f = scr.tile([P, FD], mybir.dt.int32, tag="i32t")
                        i32t = i32t_f[:, :sz]
                        nc.vector.tensor_scalar(
                            out=i32t, in0=vt, scalar1=FE_A, scalar2=FE_B,
                            op0=Alu.mult, op1=Alu.add,
                        )
                        dummy_f = scr.tile([P, FD], mybir.dt.float32, tag="fedum")
                        dummy = dummy_f[:, :sz]
                        nc.vector.tensor_scalar(
                            out=dummy, in0=i32t.bitcast(mybir.dt.float32),
                            scalar1=0.0, scalar2=1.0, op0=Alu.max, op1=Alu.mult,
                            accum_out=parts[:, ci : ci + 1],
                        )
                    else:
                        # scratch only (never read): ACT is serial anyway
                        et_f = scr.tile([P, FD], mybir.dt.float32, tag="et")
                        et = et_f[:, :sz]
                        # e = exp(16*v - 116), row-accumulated into parts[:, ci]
                        nc.scalar.activation(
                            et, vt, Act.Exp, bias=bias_t[:, :], scale=SCALE,
                            accum_out=parts[:, ci : ci + 1],
                        )
                    off += sz
                nc.vector.tensor_reduce(
                    out=stats[:, rb : rb + 1], in_=parts,
                    axis=mybir.AxisListType.X, op=Alu.add,
                )
            nc.sync.dma_start(out=out[:, :], in_=stats)
    nc.finalize()
    return nc


def _make_lut():
    m = np.arange(32768, dtype=np.int32)
    blk = m & ~15
    r = m & 15
    w_easy = np.where(r <= 3, m, np.where(r <= 9, blk + 3, blk + 16))
    w_hard = np.where(r >= 12, m, np.where((r <= 5) & (blk > 0), blk - 1, blk + 12))
    # keep v = u*q out of the fast-exp -NaN window (u in ~[1.705, 1.749]):
    # snap u in [1.66, 1.76] down to 1.5 / 1.51171875 (exp terms there are
    # ~e^-88, numerically irrelevant either way)
    val_e = w_easy.astype(np.uint16).view(np.float16).astype(np.float32)
    val_h = w_hard.astype(np.uint16).view(np.float16).astype(np.float32)
    w_easy = np.where((val_e >= 1.66) & (val_e <= 1.76), 0x3E00, w_easy)
    w_hard = np.where((val_h >= 1.66) & (val_h <= 1.76), 0x3E0C, w_hard)
    return np.concatenate([w_easy, w_hard]).astype(np.uint16)


_LUT = _make_lut()


def _encode(u_f32, pos):
    """fp16 values nearest to u with magnitude-mantissa bits 2-3 equal to
    0b11 (pos) / 0b00 (not pos); bits 0-1 stay free."""
    v = np.asarray(u_f32, dtype=np.float16).view(np.uint16)
    sign = v & np.uint16(0x8000)
    idx = (v & np.uint16(0x7FFF)).astype(np.int32)
    idx += np.where(pos > 0.5, np.int32(32768), np.int32(0))
    return (sign | _LUT[idx]).view(np.float16)


def _fastexp_host(v16):
    """Replicate the device fast-exp for fp16 v: f32 terms."""
    i = (v16.astype(np.float32) * np.float32(FE_A) + np.float32(FE_B))
    i32 = i.astype(np.int32)
    return np.maximum(i32.view(np.float32), 0.0).astype(np.float64)


def _run_device(inp, pos, trace=False):
    """Run the SPMD kernel; returns (s_dev[B] f32 row sums, exec_time_ns|None).

    inp/pos are the FULL (B, C) float32 arrays."""
    from concourse.bass_utils import run_bass_kernel_spmd

    if "nc" not in _CACHE:
        _CACHE["nc"] = _build()
    nc = _CACHE["nc"]

    u_enc = _encode(inp + np.float32(1.0), pos)

    in_maps = []
    for i in range(N_CORES):
        sl = slice(i * ROWS, (i + 1) * ROWS)
        in_maps.append({"x": np.ascontiguousarray(u_enc[sl])})
    res = run_bass_kernel_spmd(nc, in_maps, core_ids=list(range(N_CORES)), trace=trace)
    # out[p, rb] holds the sum for local row rb*128+p
    s = np.concatenate([r["out"].T.reshape(-1) for r in res.results])
    return s.astype(np.float32), res.exec_time_ns


def _fastexp_cols():
    """Global column ranges + local row-block of the fast-exp chunks."""
    rngs = []
    for rb, ci_target in FASTEXP_CHUNKS:
        off = 0
        for ci, sz in enumerate(RB_CHUNKS[rb]):
            if ci == ci_target:
                rngs.append((rb, off, off + sz))
                break
            off += sz
    return rngs


def kernel(**inputs):
    inp = np.ascontiguousarray(np.asarray(inputs["inputs"], dtype=np.float32))
    targets = np.asarray(inputs["targets"]).astype(np.int64)
    pos = np.ascontiguousarray(np.asarray(inputs["pos_cam_mask"], dtype=np.float32))

    s_dev, _ = _run_device(inp, pos)

    rows = np.arange(B)
    gt = inp[rows, targets].astype(np.float64)
    pos_t = pos[rows, targets]
    # Remove the device's term at the target column (replicating the
    # device's fp16 encode + rounding exactly), add the true one.
    u_t = _encode((gt + 1.0).astype(np.float32), pos_t)
    q_t = np.where(pos_t > 0.5, np.float16(Q_HARD), np.float16(1.0))
    v_t = (u_t * q_t).astype(np.float16)
    m_exp = np.exp(SCALE * v_t.astype(np.float64) + BIAS)
    m_fast = _fastexp_host(v_t)
    # which rows' target column landed in a fast-exp chunk?
    local_rb = (rows % ROWS) // P
    use_fast = np.zeros(B, dtype=bool)
    for rb, c0, c1 in _fastexp_cols():
        use_fast |= (local_rb == rb) & (targets >= c0) & (targets < c1)
    m_t = np.where(use_fast, m_fast, m_exp)
    corr = np.exp(SCALE * (gt - MARGIN) - K)
    s = s_dev.astype(np.float64) - m_t + corr
    loss_i = K + np.log(s) - SCALE * (gt - MARGIN)
    return np.float32(loss_i.mean())
